# revision 1
# baseline (speedup 1.0000x reference)
"""Causal Laplacian linear attention — TRN2 Bass kernel (8-core SPMD, head-sharded).

Core h handles head h for both batches (pair0 = batch0, pair1 = batch1), so the
whitening stats (mean/var over batch+query) are core-local; no collectives.

Per-core pipeline:
  P1  QKV projection (f32r matmuls; batch1 tiles land via copy+DMA lane shift)
  P2  v transpose (PE)                      -> v natural [S,64] per pair
  P3  pairwise L1 per query i:
        ACT queries: slab = |k - q_i| (Abs with bias), 1 PE ones-matmul
        DVE queries: slab_a = relu(k - q_i), slab_b = relu(q_i - k)
                     (two tensor_scalar sub+max), 2 PE ones-matmuls
      PE accumulates d-sums into [64,S] psum tiles (32 queries each, pair0 in
      rows 0:32, pair1 in 32:64); ACT exp(-l1/4); DMA deinterleave -> Wn;
      DVE tril mask in place.
  P4  PE transpose Wn -> Wt (f32r)
  P5  bn_stats/bn_aggr stats per key over (pair, query); NS norm scalars
  P6  Newton-Schulz x5: Y=W@X (PE, lhsT=Wt), Z=2I-Y (DVE), X'=X@Z (PE,
      lhsT=Xt), Xt' = PE-transpose(X')  (all f32r)
  P7  whitening of Wt -> Wnorm^T (f32r)
  P8  ctx = W^T v, c2 = W_inv ctx (fp32), gaT = (Wnorm c2)^T, out = ga @ projT

Host: shards weights per head, transposes x, sums the 8 partial outputs + bias.
"""

import numpy as np

import concourse.bass as bass
import concourse.bacc as bacc
import concourse.mybir as mybir
from concourse.tile import TileContext

F32 = mybir.dt.float32
F32R = mybir.dt.float32r
AX = mybir.AxisListType
OP = mybir.AluOpType
ACTF = mybir.ActivationFunctionType

S = 512
DIM = 512
HD = 64
B = 2
NB = S // 128
LAM = 4.0
NS_ITERS = 5
EPS = 1e-5
QG = S // 64
N_SLAB = 6
_REPS = 1
DVE_MOD = 2          # queries with i % DVE_MOD == DVE_MOD-1 take the DVE path


def build_nc(num_devices=8, reps=1):
    # reps > 1 (timing builds) cannot reuse the xT slots for Wn (cross-rep
    # WAR cycle), so shrink the slab/wiv rotation instead to stay in SBUF.
    global _REPS
    _REPS = reps
    nc = bacc.Bacc("TRN2", target_bir_lowering=False, debug=False,
                   num_devices=num_devices)

    xT_d = nc.dram_tensor("xT", [DIM, B * S], F32, kind="ExternalInput")
    wT_d = nc.dram_tensor("wT", [DIM, 3 * HD], F32, kind="ExternalInput")
    bqkv_d = nc.dram_tensor("bqkv", [128, 3], F32, kind="ExternalInput")
    projT_d = nc.dram_tensor("projT", [HD, DIM], F32, kind="ExternalInput")
    out_d = nc.dram_tensor("out", [B * S, DIM], F32, kind="ExternalOutput")
    sbc_d = nc.dram_tensor("sbc_scratch", [B, 1], F32, kind="Internal")

    tril = np.tril(np.ones((S, S), np.float32))
    twoI = (2.0 * np.eye(S)).astype(np.float32)
    eye128 = np.eye(128, dtype=np.float32)
    tril_d = nc.inline_tensor(tril, "tril")
    twoI_d = nc.inline_tensor(twoI, "twoI")
    eye_d = nc.inline_tensor(eye128, "eye128")

    with TileContext(nc) as tc:
        with (
            tc.tile_pool(name="persist", bufs=1) as pp,
            tc.tile_pool(name="roll", bufs=2) as rp,
            tc.tile_pool(name="psA", bufs=4, space="PSUM") as psA,
            tc.tile_pool(name="psB", bufs=2, space="PSUM") as psB,
            tc.tile_pool(name="psC", bufs=2, space="PSUM") as psC,
            tc.tile_pool(name="xn", bufs=2) as xn_pool,
            tc.tile_pool(name="zn", bufs=1) as zn_pool,
        ):
            # ---------- loads + constants ----------
            xT = [pp.tile([128, B * S], F32, tag=f"xTf{k}", name=f"xT{k}")
                  for k in range(NB)]
            xTr = [pp.tile([128, B * S], F32R, tag=f"xTr{k}", name=f"xTr{k}")
                   for k in range(NB)]
            for k in range(NB):
                nc.sync.dma_start(out=xT[k], in_=xT_d.ap()[k * 128:(k + 1) * 128])
                nc.vector.tensor_copy(xTr[k], xT[k])
            wT = [pp.tile([128, 3 * HD], F32, name=f"wT{k}") for k in range(NB)]
            wTr = [pp.tile([128, 3 * HD], F32R, name=f"wTr{k}") for k in range(NB)]
            for k in range(NB):
                nc.sync.dma_start(out=wT[k], in_=wT_d.ap()[k * 128:(k + 1) * 128])
                nc.vector.tensor_copy(wTr[k], wT[k])
            bqkv = pp.tile([128, 3], F32, name="bqkv")
            nc.sync.dma_start(out=bqkv, in_=bqkv_d.ap())
            projT = pp.tile([HD, DIM], F32, name="projT")
            nc.sync.dma_start(out=projT, in_=projT_d.ap())
            projT_r = pp.tile([HD, DIM], F32R, name="projT_r")
            nc.vector.tensor_copy(projT_r, projT)

            tril_s = [pp.tile([128, S], F32, name=f"tril{m}") for m in range(NB)]
            for m in range(NB):
                nc.sync.dma_start(out=tril_s[m], in_=tril_d.ap()[m * 128:(m + 1) * 128])
            twoI_s = [pp.tile([128, S], F32, name=f"twoI{m}") for m in range(NB)]
            for m in range(NB):
                nc.sync.dma_start(out=twoI_s[m], in_=twoI_d.ap()[m * 128:(m + 1) * 128])
            # reduce matrix: slot u -> col u (pair0 partitions 0-63) and
            # col 32+u (pair1 partitions 64-127)
            ones_r = pp.tile([128, 32, 64], F32R, name="onesr")
            nc.gpsimd.memset(ones_r.bitcast(F32), 0.0)
            nc.gpsimd.affine_select(
                out=ones_r[0:64], in_=ones_r[0:64], compare_op=OP.not_equal,
                fill=1.0, base=0, pattern=[[1, 32], [-1, 64]], channel_multiplier=0)
            nc.gpsimd.affine_select(
                out=ones_r[64:128], in_=ones_r[64:128], compare_op=OP.not_equal,
                fill=1.0, base=32, pattern=[[1, 32], [-1, 64]], channel_multiplier=0)
            eye_s = pp.tile([128, 128], F32, name="eye_s")
            nc.sync.dma_start(out=eye_s, in_=eye_d.ap())
            eye_r = pp.tile([128, 128], F32R, name="eye_r")
            nc.vector.tensor_copy(eye_r, eye_s)
            eps_t = pp.tile([128, 1], F32, name="eps_t")
            nc.vector.memset(eps_t, EPS)

            for rep in range(reps):
                _emit_body(nc, tc, pp, rp, psA, psB, psC, xn_pool, zn_pool,
                           xTr, wTr, bqkv, projT_r, tril_s, twoI_s, ones_r,
                           eye_s, eye_r, eps_t, out_d, sbc_d, rep)
    nc.compile()
    return nc


def _emit_body(nc, tc, pp, rp, psA, psB, psC, xn_pool, zn_pool,
               xTr, wTr, bqkv, projT_r, tril_s, twoI_s, ones_r, eye_s, eye_r,
               eps_t, out_d, sbc_d, rep):
    R = f"r{rep}"
    mb = lambda m: slice(m * 128, (m + 1) * 128)

    # ---------- P1: QKV (f32r). batch0 -> rows 0:64 direct; batch1 via
    # sbuf bounce + DMA lane shift to rows 64:128 ----------
    qT = pp.tile([128, S], F32, tag="qT", name=f"qT{R}")
    kT = pp.tile([128, S], F32, tag="kT", name=f"kT{R}")
    nqT = pp.tile([128, S], F32, tag="nqT", name=f"nqT{R}")
    nkT = pp.tile([128, S], F32, tag="nkT", name=f"nkT{R}")
    vT = pp.tile([128, S], F32, tag="vT", name=f"vT{R}")
    for b in range(B):
        q_ps = psA.tile([64, S], F32, tag="a", name=f"qps{b}{R}")
        k_ps = psB.tile([64, S], F32, tag="b", name=f"kps{b}{R}")
        v_ps = psC.tile([64, S], F32, tag="c", name=f"vps{b}{R}")
        for k in range(NB):
            nc.tensor.matmul(q_ps, wTr[k][:, 0:64],
                             xTr[k][:, b * S:(b + 1) * S],
                             start=(k == 0), stop=(k == NB - 1))
            nc.tensor.matmul(k_ps, wTr[k][:, 64:128],
                             xTr[k][:, b * S:(b + 1) * S],
                             start=(k == 0), stop=(k == NB - 1))
            nc.tensor.matmul(v_ps, wTr[k][:, 128:192],
                             xTr[k][:, b * S:(b + 1) * S],
                             start=(k == 0), stop=(k == NB - 1))
        if b == 0:
            nc.scalar.activation(qT[0:64], q_ps, ACTF.Identity,
                                 bias=bqkv[0:64, 0:1], scale=1.0)
            nc.vector.tensor_scalar(nqT[0:64], qT[0:64], -1.0, None, op0=OP.mult)
            nc.scalar.activation(kT[0:64], k_ps, ACTF.Identity,
                                 bias=bqkv[0:64, 1:2], scale=1.0)
            nc.vector.tensor_scalar(nkT[0:64], kT[0:64], -1.0, None, op0=OP.mult)
            nc.scalar.activation(vT[0:64], v_ps, ACTF.Identity,
                                 bias=bqkv[0:64, 2:3], scale=1.0)
        else:
            tmp = {}
            for nm, ps, col in (("q", q_ps, 0), ("k", k_ps, 1), ("v", v_ps, 2)):
                t = pp.tile([64, S], F32, tag=f"b1{nm}", name=f"b1{nm}{R}")
                nc.scalar.activation(t, ps, ACTF.Identity,
                                     bias=bqkv[64:128, col:col + 1], scale=1.0)
                tmp[nm] = t
            nq = pp.tile([64, S], F32, tag="b1nq", name=f"b1nq{R}")
            nc.vector.tensor_scalar(nq, tmp["q"], -1.0, None, op0=OP.mult)
            nk = pp.tile([64, S], F32, tag="b1nk", name=f"b1nk{R}")
            nc.vector.tensor_scalar(nk, tmp["k"], -1.0, None, op0=OP.mult)
            nc.sync.dma_start(out=qT[64:128], in_=tmp["q"])
            nc.sync.dma_start(out=kT[64:128], in_=tmp["k"])
            nc.sync.dma_start(out=vT[64:128], in_=tmp["v"])
            nc.sync.dma_start(out=nqT[64:128], in_=nq)
            nc.sync.dma_start(out=nkT[64:128], in_=nk)

    # ---------- P2: v natural ----------
    v_nat = [[pp.tile([128, HD], F32, tag=f"vnat{p}_{t}", name=f"vnat{p}_{t}{R}")
              for t in range(NB)] for p in range(B)]
    for t in range(NB):
        vt_ps = psC.tile([128, 128], F32, tag="c", name=f"vtps{t}{R}")
        nc.tensor.transpose(vt_ps, vT[:, t * 128:(t + 1) * 128], eye_s)
        nc.scalar.copy(v_nat[0][t], vt_ps[:, 0:64])
        nc.scalar.copy(v_nat[1][t], vt_ps[:, 64:128])

    # ---------- P3: pairwise L1 ----------
    n_slab = N_SLAB if _REPS == 1 else 3
    n_wiv = 4 if _REPS == 1 else 2
    wn0_tag = (lambda m: f"xTf{m}") if _REPS == 1 else (lambda m: f"Wn0_{m}")
    Wn = [[pp.tile([128, S], F32, tag=wn0_tag(m) if p == 0 else f"Wn1_{m}",
                   name=f"Wn{p}_{m}{R}") for m in range(NB)] for p in range(B)]
    slabs = [pp.tile([128, S], F32R, tag=f"slab{s}", name=f"slab{s}{R}")
             for s in range(n_slab)]
    wivs = [pp.tile([64, S], F32, tag=f"wiv{s}", name=f"wiv{s}{R}")
            for s in range(n_wiv)]
    if rep == 0:
        for s in range(n_slab):
            nc.gpsimd.memset(slabs[s].bitcast(F32), 0.0)
        for s in range(n_wiv):
            nc.gpsimd.memset(wivs[s], 0.0)
    sl = 0
    for g in range(QG):
        wmx = max((g + 1) * 64, 256)
        l1h = [psA.tile([64, S], F32, tag="a", name=f"l1ps{g}_{h}{R}")
               for h in range(2)]
        started = [False, False]
        for t in range(64):
            i = g * 64 + t
            w = i + 1
            h = t // 32
            u = t % 32
            last = (u == 31)
            if i % 2 == 1:
                sa = slabs[sl % n_slab]; sl += 1
                sb = slabs[sl % n_slab]; sl += 1
                nc.vector.tensor_scalar(sa[:, 0:w], kT[:, 0:w],
                                        qT[:, i:i + 1], 0.0,
                                        op0=OP.subtract, op1=OP.max)
                nc.vector.tensor_scalar(sb[:, 0:w], nkT[:, 0:w],
                                        nqT[:, i:i + 1], 0.0,
                                        op0=OP.subtract, op1=OP.max)
                nc.tensor.matmul(l1h[h][:, 0:wmx], ones_r[:, u, :],
                                 sa[:, 0:wmx], start=not started[h], stop=False)
                started[h] = True
                nc.tensor.matmul(l1h[h][:, 0:wmx], ones_r[:, u, :],
                                 sb[:, 0:wmx], start=False, stop=last)
            else:
                slab = slabs[sl % n_slab]; sl += 1
                nc.scalar.activation(slab[:, 0:w], kT[:, 0:w], ACTF.Abs,
                                     bias=qT[:, i:i + 1], scale=-1.0)
                nc.tensor.matmul(l1h[h][:, 0:wmx], ones_r[:, u, :],
                                 slab[:, 0:wmx], start=not started[h], stop=last)
                started[h] = True
        m, half = g // 2, (g % 2) * 64
        for h in range(2):
            wiv = wivs[((g % 2) * 2 + h) % n_wiv]
            nc.scalar.activation(wiv[:, 0:wmx], l1h[h][:, 0:wmx],
                                 ACTF.Exp, bias=0.0, scale=-1.0 / LAM)
            nc.sync.dma_start(
                out=Wn[0][m][half + 32 * h:half + 32 * h + 32, :],
                in_=wiv[0:32, :])
            nc.sync.dma_start(
                out=Wn[1][m][half + 32 * h:half + 32 * h + 32, :],
                in_=wiv[32:64, :])
    for p in range(B):
        for m in range(NB):
            nc.vector.tensor_mul(Wn[p][m], Wn[p][m], tril_s[m])

    # ---------- P4: Wt via PE transpose ----------
    _wt0_tags = ["b1q", "b1k", "b1v", "b1nq"]
    Wt = [[pp.tile([128, S], F32R, tag=_wt0_tags[m] if p == 0 else f"Wt1_{m}",
                   name=f"Wt{p}_{m}{R}") for m in range(NB)] for p in range(B)]
    for p in range(B):
        for m in range(NB):
            tp = psB.tile([128, S], F32, tag="b", name=f"wtps{p}_{m}{R}")
            for k in range(NB):
                nc.tensor.transpose(tp[:, k * 128:(k + 1) * 128],
                                    Wn[p][k][:, m * 128:(m + 1) * 128], eye_s)
            nc.scalar.copy(Wt[p][m], tp)

    # ---------- P5: stats + NS norms ----------
    mv = []
    istd = []
    for m in range(NB):
        st = rp.tile([128, B, 6], F32, tag="bnst", name=f"bnst{m}{R}")
        for p in range(B):
            nc.vector.bn_stats(st[:, p, :], Wt[p][m].bitcast(F32))
        mvm = pp.tile([128, 2], F32, tag=f"mv{m}", name=f"mv{m}{R}")
        nc.vector.bn_aggr(mvm, st)
        sd = rp.tile([128, 1], F32, tag="sd", name=f"sd{m}{R}")
        N = B * S
        nc.scalar.activation(sd, mvm[:, 1:2], ACTF.Sqrt,
                             bias=eps_t, scale=float(N) / (N - 1))
        ist = pp.tile([128, 1], F32, tag=f"istd{m}", name=f"istd{m}{R}")
        nc.vector.reciprocal(ist, sd)
        mv.append(mvm)
        istd.append(ist)

    s_bc = []
    for p in range(B):
        rs = rp.tile([128, 2 * NB], F32, tag="rs", name=f"rs{p}{R}")
        for m in range(NB):
            nc.vector.tensor_reduce(rs[:, m:m + 1], Wt[p][m].bitcast(F32),
                                    axis=AX.X, op=OP.add)
            nc.vector.tensor_reduce(rs[:, NB + m:NB + m + 1], Wn[p][m],
                                    axis=AX.X, op=OP.add)
        rc = rp.tile([1, 2 * NB], F32, tag="rc", name=f"rc{p}{R}")
        nc.gpsimd.tensor_reduce(rc, rs, axis=AX.C, op=OP.max)
        n1 = rp.tile([1, 1], F32, tag="n1", name=f"n1{p}{R}")
        ninf = rp.tile([1, 1], F32, tag="ninf", name=f"ninf{p}{R}")
        nc.vector.tensor_reduce(n1, rc[:, 0:NB], axis=AX.X, op=OP.max)
        nc.vector.tensor_reduce(ninf, rc[:, NB:2 * NB], axis=AX.X, op=OP.max)
        pr = rp.tile([1, 1], F32, tag="pr", name=f"pr{p}{R}")
        nc.vector.tensor_mul(pr, n1, ninf)
        inv = rp.tile([1, 1], F32, tag="inv", name=f"inv{p}{R}")
        nc.vector.reciprocal(inv, pr)
        nc.sync.dma_start(out=sbc_d.ap()[p:p + 1, :], in_=inv)
        sb2 = pp.tile([128, 1], F32, tag=f"sbc{p}", name=f"sbc{p}{R}")
        nc.sync.dma_start(out=sb2, in_=bass.AP(
            tensor=sbc_d, offset=p, ap=[[0, 128], [1, 1]]))
        s_bc.append(sb2)

    # ---------- P6: Newton-Schulz ----------
    Xn = [[None] * NB for _ in range(B)]
    Xt = [[None] * NB for _ in range(B)]
    for p in range(B):
        for m in range(NB):
            x0 = xn_pool.tile([128, S], F32R, tag=f"xn{p}{m}", name=f"x0_{p}_{m}{R}")
            nc.vector.tensor_scalar(x0, Wt[p][m], s_bc[p], None, op0=OP.mult)
            Xn[p][m] = x0
            x0t = xn_pool.tile([128, S], F32R, tag=f"xt{p}{m}", name=f"x0t_{p}_{m}{R}")
            nc.vector.tensor_scalar(x0t, Wn[p][m], s_bc[p], None, op0=OP.mult)
            Xt[p][m] = x0t

    # whitening + context: independent of X, emitted early so the
    # scheduler can fill NS-phase engine gaps with them
    _wnt_extra = (["Wnt_x0", "Wnt_x1", "Wnt_x2", "Wnt_x3"] if _REPS == 1
                  else ["qT", "kT", "nqT", "nkT"])
    _wnt_tags = ([f"slab{s}" for s in range(n_slab)]
                 + [f"wiv{s}" for s in range(n_wiv)] + _wnt_extra)[:8]
    Wnt = [[pp.tile([128, S], F32R, tag=_wnt_tags[p * NB + m],
                    name=f"Wnt{p}_{m}{R}") for m in range(NB)] for p in range(B)]
    for p in range(B):
        for m in range(NB):
            nc.vector.tensor_scalar(Wnt[p][m], Wt[p][m], mv[m][:, 0:1],
                                    istd[m], op0=OP.subtract, op1=OP.mult)
    ctx_all = []
    for p in range(B):
        ctx = [None] * NB
        for m in range(NB):
            cps = psA.tile([128, HD], F32, tag="a", name=f"cps{p}_{m}{R}")
            for k in range(m, NB):          # W[j,i]=0 for j<i: skip k<m blocks
                nc.tensor.matmul(cps, Wn[p][k][:, mb(m)], v_nat[p][k],
                                 start=(k == m), stop=(k == NB - 1))
            ct = rp.tile([128, HD], F32, tag=f"ctx{p}{m}", name=f"ctx{p}_{m}{R}")
            nc.scalar.copy(ct, cps)
            ctx[m] = ct
        ctx_all.append(ctx)

    for it in range(NS_ITERS):
        for p in range(B):
            Zn = [None] * NB
            for m in range(NB):
                y_ps = psA.tile([128, S], F32, tag="a", name=f"yps{it}_{p}_{m}{R}")
                for k in range(m + 1):      # W lower-triangular: k>m blocks are 0
                    nc.tensor.matmul(y_ps, Wt[p][k][:, mb(m)], Xn[p][k],
                                     start=(k == 0), stop=(k == m))
                _zn_tags = [["tril0", "tril1", "tril2", "tril3"],
                            ["qT", "kT", "nqT", "nkT"]]
                z = pp.tile([128, S], F32R, tag=_zn_tags[p][m],
                            name=f"z{it}_{p}_{m}{R}")
                nc.vector.tensor_tensor(z, twoI_s[m], y_ps, op=OP.subtract)
                Zn[m] = z
            Xn_new = [None] * NB
            for m in range(NB):
                xps = psB.tile([128, S], F32, tag="b", name=f"xps{it}_{p}_{m}{R}")
                k_lo = m if it == 0 else 0   # X0 upper-tri: k<m blocks are 0
                for k in range(k_lo, NB):
                    nc.tensor.matmul(xps, Xt[p][k][:, mb(m)], Zn[k],
                                     start=(k == k_lo), stop=(k == NB - 1))
                xn1 = xn_pool.tile([128, S], F32R, tag=f"xn{p}{m}",
                                   name=f"xn{it}_{p}_{m}{R}")
                nc.scalar.copy(xn1, xps)
                Xn_new[m] = xn1
            Xt_new = [None] * NB
            for m in range(NB):
                xtps = psC.tile([128, S], F32, tag="c", name=f"xtps{it}_{p}_{m}{R}")
                for k in range(NB):
                    nc.tensor.transpose(xtps[:, k * 128:(k + 1) * 128],
                                        Xn_new[k].bitcast(F32)[:, mb(m)], eye_s)
                xt1 = xn_pool.tile([128, S], F32R, tag=f"xt{p}{m}",
                                   name=f"xt{it}_{p}_{m}{R}")
                nc.vector.tensor_copy(xt1, xtps)
                Xt_new[m] = xt1
            Xn[p] = Xn_new
            Xt[p] = Xt_new

    # ---------- P8: tail ----------
    for p in range(B):
        ctx = ctx_all[p]
        c2 = [None] * NB
        for m in range(NB):
            c2ps = psB.tile([128, HD], F32, tag="b", name=f"c2ps{p}_{m}{R}")
            for k in range(NB):
                nc.tensor.matmul(c2ps, Xt[p][k].bitcast(F32)[:, mb(m)],
                                 ctx[k], start=(k == 0), stop=(k == NB - 1))
            c2t = rp.tile([128, HD], F32R, tag=f"c2{p}{m}", name=f"c2{p}_{m}{R}")
            nc.scalar.copy(c2t, c2ps)
            c2[m] = c2t
        ga_ps = psC.tile([64, S], F32, tag="c", name=f"gaps{p}{R}")
        for k in range(NB):
            nc.tensor.matmul(ga_ps, c2[k], Wnt[p][k],
                             start=(k == 0), stop=(k == NB - 1))
        gaT = rp.tile([64, S], F32R, tag="gaT", name=f"gaT{p}{R}")
        nc.scalar.copy(gaT, ga_ps)
        for m in range(NB):
            ops = psA.tile([128, S], F32, tag="a", name=f"ops{p}_{m}{R}")
            nc.tensor.matmul(ops, gaT[:, mb(m)], projT_r, start=True, stop=True)
            ob = rp.tile([128, S], F32, tag="ob", name=f"ob{p}_{m}{R}")
            nc.scalar.copy(ob, ops)
            nc.sync.dma_start(
                out=out_d.ap()[p * S + m * 128: p * S + (m + 1) * 128], in_=ob)


def host_prep(inputs):
    x = np.ascontiguousarray(inputs["x"], np.float32)
    qkv_w = np.ascontiguousarray(inputs["qkv_w"], np.float32)
    qkv_b = np.ascontiguousarray(inputs["qkv_b"], np.float32)
    proj_w = np.ascontiguousarray(inputs["proj_w"], np.float32)
    xT = np.ascontiguousarray(x.reshape(B * S, DIM).T)
    in_maps = []
    for h in range(8):
        wq = qkv_w[h * HD:(h + 1) * HD]
        wk = qkv_w[DIM + h * HD:DIM + (h + 1) * HD]
        wv = qkv_w[2 * DIM + h * HD:2 * DIM + (h + 1) * HD]
        wT = np.ascontiguousarray(np.concatenate([wq, wk, wv], axis=0).T)
        bq = qkv_b[h * HD:(h + 1) * HD]
        bk = qkv_b[DIM + h * HD:DIM + (h + 1) * HD]
        bv = qkv_b[2 * DIM + h * HD:2 * DIM + (h + 1) * HD]
        bqkv = np.ascontiguousarray(
            np.stack([np.concatenate([bq, bq]), np.concatenate([bk, bk]),
                      np.concatenate([bv, bv])], axis=1))
        projT = np.ascontiguousarray(proj_w[:, h * HD:(h + 1) * HD].T)
        in_maps.append({"xT": xT, "wT": wT, "bqkv": bqkv, "projT": projT})
    return in_maps


def host_finish(results, inputs):
    proj_b = np.asarray(inputs["proj_b"], np.float32)
    acc = np.zeros((B * S, DIM), np.float32)
    for r in results:
        acc += r["out"]
    return (acc + proj_b).reshape(B, S, DIM)


# ----------------------------------------------------------------------------
# Harness entry point: full inputs in, full output out.
# Sharding: core h computes head h for both batches (B*H = 16 (b,h) pairs ->
# 2 per core); host sums the 8 per-head partial outputs (the output projection
# is a sum over heads) and adds proj_b.
# ----------------------------------------------------------------------------
_NC_CACHE = {}


def _get_nc():
    if "nc" not in _NC_CACHE:
        _NC_CACHE["nc"] = build_nc(8)
    return _NC_CACHE["nc"]


def kernel(**inputs):
    from concourse.bass_utils import run_bass_kernel_spmd
    nc = _get_nc()
    in_maps = host_prep(inputs)
    res = run_bass_kernel_spmd(nc, in_maps, core_ids=list(range(8)))
    return host_finish(res.results, inputs)

